# revision 1
# baseline (speedup 1.0000x reference)
"""Trainium2 Bass kernel for nn_MultiHeadAttention (B=2, S=2048, D=1024, H=16, causal).

Sharding across 8 NeuronCores (single SPMD program):
  - Core c owns batch b=c//4 and two 256-token query chunks {p, 7-p} (p=c%4);
    the pairing balances causal attention work.
  - Everything on-chip is bf16 (PSUM accumulation stays fp32): halves HBM +
    collective bytes vs fp32 and enables fast weight loads (FWL) on the PE.
  - Phase 1: project K^T, V (with softmax scale folded into Wk/bk on the
    host), publish both with ONE AllGather (replica groups [[0-3],[4-7]]);
    Q projection + Wo load overlap the collective.  All biases are applied
    with rank-1 matmuls into PSUM (no scalar-engine bias pass).
  - Phase 2: K^T and V for the whole batch live in SBUF.  Heads are
    processed in pairs (feature block = 128 partitions); per key block the
    two heads' score matmuls are row-tiled (partitions 0:64 / 64:128) into
    one 2-bank PSUM tile so they run concurrently, one wide exp covers both,
    causal masking is a single multiplicative bf16 DVE op on the (host
    per-core) staircase, and ctx accumulates in PSUM across ALL 16 key
    blocks (65th stationary column = softmax denominator).  The softmax
    reciprocal is exp(-ln(d)) on the scalar engine (DVE reciprocal is
    8 cyc/elem and was 67us in the fp32 baseline).
  - Phase 3: output projection for the core's own tokens (row-parallel over
    tokens => no reduction); host re-assembles the full output.
"""
import numpy as np
import ml_dtypes

import concourse.bass as bass
import concourse.bacc as bacc
import concourse.mybir as mybir
import concourse.tile as tile
from concourse.bass_utils import run_bass_kernel_spmd
from concourse.tile_rust import add_dep_helper

B, S, D, H, HD = 2, 2048, 1024, 16, 64
NC = 8
P = 128
F32 = mybir.dt.float32
BF = mybir.dt.bfloat16
NPBF = ml_dtypes.bfloat16

KT_N = D * 512           # K^T shard elems  [1024, 512]
V_N = 512 * 16 * 65      # V shard elems    [512 tok, 16 heads, 64+1]
KV_N = KT_N + V_N

TRACE = False        # set True (e.g. from test.py) to capture an NTFF profile
LAST_RESULT = None   # BassKernelResults of the most recent kernel() call

_ACT_PATCHED = False


def _patch_act_tables():
    """Steer Bacc's act-table-load pass to the combined natural_log+exp
    set.  The pass assigns each activation function the FIRST table set
    containing it, so a kernel using both Exp and Ln alternates between
    `exp_and_others` and `natural_log` -- one ~1.3us ACT_TABLE_LOAD per
    transition (17 loads / 22us on the scalar engine for this kernel).
    Hiding Exp/Ln from the earlier sets makes both resolve to the single
    `natural_log_exp_and_others` set (one load total).  List length and
    order are preserved, so the set ids walrus emits stay valid."""
    global _ACT_PATCHED
    if _ACT_PATCHED:
        return
    import concourse.bacc as _bacc
    _orig = _bacc.get_activation_tables

    def _filtered(arch):
        t = _orig(arch)
        fexp = mybir.ActivationFunctionType.Exp
        fln = mybir.ActivationFunctionType.Ln
        out = {}
        for name, fns in t.items():
            if name != "natural_log_exp_and_others" and (
                    fexp in fns or fln in fns):
                fns = fns - {fexp, fln}
            out[name] = fns
        return out

    _bacc.get_activation_tables = _filtered
    _ACT_PATCHED = True


def sel_tokens(p):
    return list(range(256 * p, 256 * p + 256)) + list(
        range(256 * (7 - p), 256 * (7 - p) + 256)
    )


def _kblk(j):
    """Original 128-token key block j -> (rank-in-group, column offset)."""
    q = j // 2
    rr = q if q <= 3 else 7 - q
    off = (0 if q <= 3 else 256) + 128 * (j % 2)
    return rr, off


def _emit(causal: bool, repeat: int = 1):
    nc = bacc.Bacc(trn_type="TRN2", num_devices=NC)
    fexp = mybir.ActivationFunctionType.Exp
    fln = mybir.ActivationFunctionType.Ln
    _patch_act_tables()

    xT = nc.dram_tensor("xT", [D, 512], BF, kind="ExternalInput")
    wqT = nc.dram_tensor("wqT", [D, D], BF, kind="ExternalInput")
    wkT = nc.dram_tensor("wkT", [D, D], BF, kind="ExternalInput")
    wvT = nc.dram_tensor("wvT", [D, D], BF, kind="ExternalInput")
    woT = nc.dram_tensor("woT", [D, D], BF, kind="ExternalInput")
    bq_d = nc.dram_tensor("bq", [1, D], BF, kind="ExternalInput")
    bk_d = nc.dram_tensor("bk", [1, D], BF, kind="ExternalInput")
    bv_d = nc.dram_tensor("bv", [1, D], BF, kind="ExternalInput")
    bo_d = nc.dram_tensor("bo", [1, D], BF, kind="ExternalInput")
    if causal:
        cmb_d = nc.dram_tensor("cmb", [P, 16, 2, 256], BF, kind="ExternalInput")
    outT = nc.dram_tensor("outT", [D, 512], F32, kind="ExternalOutput")

    kv_loc = nc.dram_tensor("kv_loc", [KV_N], BF)
    kv_all = nc.dram_tensor("kv_all", [4, KV_N], BF)

    with tile.TileContext(nc) as tc, \
         tc.tile_pool(name="const", bufs=1) as const, \
         tc.tile_pool(name="w", bufs=2) as wpool, \
         tc.tile_pool(name="big", bufs=1) as big, \
         tc.tile_pool(name="kv", bufs=1) as kvp, \
         tc.tile_pool(name="io", bufs=3) as io, \
         tc.tile_pool(name="vio", bufs=2) as vio, \
         tc.tile_pool(name="oio", bufs=2) as oio, \
         tc.tile_pool(name="ex", bufs=4) as ex, \
         tc.tile_pool(name="sm", bufs=2) as sm, \
         tc.tile_pool(name="ps_sc", bufs=2, space="PSUM") as ps_sc, \
         tc.tile_pool(name="ps_ctx", bufs=2, space="PSUM") as ps_ctx, \
         tc.tile_pool(name="ps_w", bufs=2, space="PSUM") as ps_w:

        # ---------- constants ----------
        ones = const.tile([P, 512], BF)
        nc.gpsimd.memset(ones[:], 1.0)
        bq_sb = const.tile([1, D], BF)
        nc.sync.dma_start(bq_sb[:], bq_d[:])
        bk_sb = const.tile([1, D], BF)
        nc.sync.dma_start(bk_sb[:], bk_d[:])
        bv_sb = const.tile([1, D], BF)
        nc.sync.dma_start(bv_sb[:], bv_d[:])
        bo_sb = const.tile([1, D], BF)
        nc.sync.dma_start(bo_sb[:], bo_d[:])
        if causal:
            cmb_sb = big.tile([P, 16, 2, 256], BF)

        rg = [[0, 1, 2, 3], [4, 5, 6, 7]]
        kt_ap = kv_loc[0:KT_N].rearrange("(o p t) -> p o t", o=8, p=P, t=512)
        v_ap = kv_loc[KT_N:KV_N].rearrange("(a p h c) -> p a h c",
                                           a=4, p=P, h=16, c=65)

        for _rep in range(repeat):
            # ---------- phase 1: projections for this core's 512 tokens ----
            xt_sb = big.tile([P, 8, 512], BF)
            xr = xT.rearrange("(o p) t -> p o t", p=P)
            for _kt in range(8):
                nc.sync.dma_start(xt_sb[:, _kt, :], xr[:, _kt, :])
            qt_sb = big.tile([P, 8, 512], BF)

            def load_w(w_dram):
                w_sb = wpool.tile([P, 8, D], BF, tag="w")
                wr = w_dram.rearrange("(o p) t -> p o t", p=P)
                for _kt in range(8):
                    nc.sync.dma_start(w_sb[:, _kt, :], wr[:, _kt, :])
                return w_sb

            def proj_T(w_sb, bias_sb, sink):
                # out[feat, tok]: per-partition bias via rank-1 matmul
                for dt in range(8):
                    pt = ps_w.tile([P, 512], F32, tag="psw")
                    for kt in range(8):
                        nc.tensor.matmul(
                            pt[:], w_sb[:, kt, 128 * dt:128 * dt + 128],
                            xt_sb[:, kt, :], start=(kt == 0), stop=False)
                    nc.tensor.matmul(
                        pt[:], bias_sb[0:1, 128 * dt:128 * dt + 128],
                        ones[0:1, 0:512], start=False, stop=True)
                    sink(dt, pt)

            # K^T -> kv_loc
            wk_sb = load_w(wkT)

            def k_sink(dt, pt):
                t = io.tile([P, 512], BF, tag="io")
                nc.vector.tensor_copy(t[:], pt[:])
                nc.sync.dma_start(kt_ap[:, dt, :], t[:])
            proj_T(wk_sb, bk_sb, k_sink)

            # V -> kv_loc ([tok, head, 64] + ones column)
            wv_sb = load_w(wvT)
            for st in range(4):
                vt = vio.tile([P, 16, 65], BF, tag="vio")
                for hf in range(2):
                    pt = ps_w.tile([P, 512], F32, tag="psw")
                    for kt in range(8):
                        nc.tensor.matmul(
                            pt[:], xt_sb[:, kt, 128 * st:128 * st + 128],
                            wv_sb[:, kt, 512 * hf:512 * hf + 512],
                            start=(kt == 0), stop=False)
                    nc.tensor.matmul(
                        pt[:], ones[0:1, 0:P],
                        bv_sb[0:1, 512 * hf:512 * hf + 512],
                        start=False, stop=True)
                    nc.vector.tensor_copy(
                        vt[:, 8 * hf:8 * hf + 8, 0:64],
                        pt[:].rearrange("p (h d) -> p h d", h=8))
                nc.vector.tensor_copy(vt[:, :, 64:65], ones[:, 0:16, None])
                nc.sync.dma_start(v_ap[:, st, :, :], vt[:])

            # ONE AllGather for K^T + V
            cc = nc.gpsimd.collective_compute(
                "AllGather", mybir.AluOpType.bypass, replica_groups=rg,
                ins=[kv_loc[:]], outs=[kv_all[:]])

            # mask load deferred to here: first needed by attention, and at
            # t=0 its 2MB competed with the x/weight DMAs feeding the PE
            if causal:
                nc.sync.dma_start(cmb_sb[:], cmb_d[:])

            # Q (stays in SBUF, bf16) -- overlaps the collective
            wq_sb = load_w(wqT)

            def q_sink(dt, pt):
                nc.vector.tensor_copy(qt_sb[:, dt, :], pt[:])
            proj_T(wq_sb, bq_sb, q_sink)
            wo_sb = load_w(woT)

            # stage gathered K^T / V into SBUF
            kt_sb = kvp.tile([P, 4, 8, 512], BF)
            v_sb = kvp.tile([P, 16, 16, 65], BF)
            for r in range(4):
                src = kv_all[r, 0:KT_N].rearrange("(o p t) -> p o t",
                                                  o=8, p=P, t=512)
                dk = nc.sync.dma_start(kt_sb[:, r, :, :], src[:, :, :])
                add_dep_helper(dk.ins, cc.ins, reason="read after AG")
                vsrc = kv_all[r, KT_N:KV_N].rearrange(
                    "(a p h c) -> p a h c", a=4, p=P, h=16, c=65)
                for a in range(4):
                    dv = nc.sync.dma_start(v_sb[:, 4 * r + a, :, :],
                                           vsrc[:, a, :, :])
                    add_dep_helper(dv.ins, cc.ins, reason="read after AG")

            # ---------- phase 2: attention, head pairs ----------
            ctx_sb = big.tile([P, 8, 512], BF)
            for pair in range(8):
                h0, h1 = 2 * pair, 2 * pair + 1
                ctx0 = ps_ctx.tile([P, 512], F32, tag="ctx")
                ctx1 = ps_ctx.tile([P, 512], F32, tag="ctx")
                for j in range(16):
                    wid = 512 if (not causal or j < 8) else 256
                    qoff = 0 if (not causal or j < 8) else 256
                    rr, off = _kblk(j)
                    sc = ps_sc.tile([P, 1024], F32, tag="sc")
                    # two heads row-tiled: run concurrently on the PE
                    nc.tensor.matmul(
                        sc[:, 0:wid],
                        kt_sb[0:64, rr, pair, off:off + 128],
                        qt_sb[0:64, pair, qoff:qoff + wid],
                        start=True, stop=True)
                    nc.tensor.matmul(
                        sc[:, 512:512 + wid],
                        kt_sb[64:128, rr, pair, off:off + 128],
                        qt_sb[64:128, pair, qoff:qoff + wid],
                        start=True, stop=True)
                    et = ex.tile([P, 2, 512], BF, tag="exp")
                    if wid == 512:
                        nc.scalar.activation(et[:, :, :], sc[:, :], fexp)
                    else:
                        nc.scalar.activation(
                            et[:, :, 0:256],
                            sc[:].rearrange("p (s n) -> p s n", s=2)
                            [:, :, 0:256], fexp)
                    if causal:
                        nc.vector.tensor_tensor(
                            et[:, :, 0:256], et[:, :, 0:256],
                            cmb_sb[:, j, :, :], mybir.AluOpType.mult)
                    vj = 4 * rr + off // 128   # v_sb slot (rank-major order)
                    nc.tensor.matmul(
                        ctx0[0:65, qoff:qoff + wid],
                        v_sb[:, vj, h0, 0:65], et[:, 0, 0:wid],
                        start=(j == 0), stop=(j == 15))
                    nc.tensor.matmul(
                        ctx1[0:65, qoff:qoff + wid],
                        v_sb[:, vj, h1, 0:65], et[:, 1, 0:wid],
                        start=(j == 0), stop=(j == 15))
                # normalize: recip = exp(-ln(denominator)); the two heads'
                # chains are interleaved stage-by-stage so the ACT/DVE ops
                # pipeline and the ctx banks free sooner.  Even head lands
                # on partitions 0:64 of ctx_sb, odd head is moved to 64:128
                # with a small SBUF->SBUF DMA (cross-partition).
                lnd0 = sm.tile([1, 512], F32, tag="lnd")
                nc.scalar.activation(lnd0[:], ctx0[64:65, 0:512], fln)
                lnd1 = sm.tile([1, 512], F32, tag="lnd")
                nc.scalar.activation(lnd1[:], ctx1[64:65, 0:512], fln)
                rcp0 = sm.tile([1, 512], BF, tag="rcp")
                nc.scalar.activation(rcp0[:], lnd0[:], fexp, scale=-1.0)
                rcp1 = sm.tile([1, 512], BF, tag="rcp")
                nc.scalar.activation(rcp1[:], lnd1[:], fexp, scale=-1.0)
                rep_ps0 = ps_w.tile([P, 512], F32, tag="psw")
                nc.tensor.matmul(rep_ps0[0:64, :], ones[0:1, 0:64],
                                 rcp0[0:1, :], start=True, stop=True)
                rep_ps1 = ps_w.tile([P, 512], F32, tag="psw")
                nc.tensor.matmul(rep_ps1[0:64, :], ones[0:1, 0:64],
                                 rcp1[0:1, :], start=True, stop=True)
                rep0 = sm.tile([64, 512], F32, tag="rep")
                nc.vector.tensor_copy(rep0[:], rep_ps0[0:64, :])
                rep1 = sm.tile([64, 512], F32, tag="rep")
                nc.vector.tensor_copy(rep1[:], rep_ps1[0:64, :])
                nc.vector.tensor_tensor(
                    ctx_sb[0:64, pair, :], ctx0[0:64, :],
                    rep0[:], mybir.AluOpType.mult)
                ctmp = sm.tile([64, 512], BF, tag="ctmp")
                nc.vector.tensor_tensor(
                    ctmp[:], ctx1[0:64, :], rep1[:],
                    mybir.AluOpType.mult)
                nc.sync.dma_start(ctx_sb[64:128, pair, :], ctmp[:])

            # ---------- phase 3: output projection ----------
            for m in range(8):
                pt = ps_w.tile([P, 512], F32, tag="psw")
                for kt in range(8):
                    nc.tensor.matmul(
                        pt[:], wo_sb[:, kt, 128 * m:128 * m + 128],
                        ctx_sb[:, kt, :], start=(kt == 0), stop=False)
                nc.tensor.matmul(
                    pt[:], bo_sb[0:1, 128 * m:128 * m + 128],
                    ones[0:1, 0:512], start=False, stop=True)
                t = oio.tile([P, 512], F32, tag="oio")
                nc.vector.tensor_copy(t[:], pt[:])
                nc.sync.dma_start(
                    outT.rearrange("(o p) t -> p o t", p=P)[:, m, :], t[:])

    nc.compile()
    return nc


_CACHE = {}


def _get_nc(causal: bool, repeat: int = 1):
    key = (causal, repeat)
    if key not in _CACHE:
        _CACHE[key] = _emit(causal, repeat)
    return _CACHE[key]


def _mask01(p):
    """Per-core multiplicative mask [128, 16, 2, 256] for the causal
    staircase (same mask for both heads of a pair, hence the dim of 2)."""
    k = np.arange(128)[:, None]
    c = np.arange(256)[None, :]
    m1 = (c - k >= 0).astype(np.float32)
    m2 = (c - 128 - k >= 0).astype(np.float32)
    cmb = np.ones((128, 16, 256), dtype=np.float32)
    # j<8: masks the LOW chunk (cols 0:256 of the 512-wide tile)
    for j in range(8):
        if j == 2 * p:
            cmb[:, j, :] = m1
        elif j == 2 * p + 1:
            cmb[:, j, :] = m2
        elif j > 2 * p + 1:
            cmb[:, j, :] = 0.0
    # j>=8: masks the HIGH chunk (the only 256 cols computed)
    for j in range(8, 16):
        if j == 14 - 2 * p:
            cmb[:, j, :] = m1
        elif j == 15 - 2 * p:
            cmb[:, j, :] = m2
        elif j > 15 - 2 * p:
            cmb[:, j, :] = 0.0
    return np.ascontiguousarray(
        np.broadcast_to(cmb[:, :, None, :], (128, 16, 2, 256))
    ).astype(NPBF)


def kernel(**inputs):
    x = np.asarray(inputs["x"], dtype=np.float32)
    Wq = np.asarray(inputs["Wq"], dtype=np.float32)
    bq = np.asarray(inputs["bq"], dtype=np.float32)
    Wk = np.asarray(inputs["Wk"], dtype=np.float32)
    bk = np.asarray(inputs["bk"], dtype=np.float32)
    Wv = np.asarray(inputs["Wv"], dtype=np.float32)
    bv = np.asarray(inputs["bv"], dtype=np.float32)
    Wo = np.asarray(inputs["Wo"], dtype=np.float32)
    bo = np.asarray(inputs["bo"], dtype=np.float32)
    causal = bool(int(np.asarray(inputs["enable_causal"])))

    scale = np.float32(1.0 / np.sqrt(HD))
    wqT = np.ascontiguousarray(Wq.T).astype(NPBF)
    wkT = np.ascontiguousarray((Wk * scale).T).astype(NPBF)
    wvT = np.ascontiguousarray(Wv.T).astype(NPBF)
    woT = np.ascontiguousarray(Wo.T).astype(NPBF)
    bqr = bq.reshape(1, D).astype(NPBF)
    bkr = (bk * scale).reshape(1, D).astype(NPBF)
    bvr = bv.reshape(1, D).astype(NPBF)
    bor = bo.reshape(1, D).astype(NPBF)

    nc = _get_nc(causal)
    in_maps = []
    for c in range(NC):
        b, p = divmod(c, 4)
        sel = sel_tokens(p)
        xTc = np.ascontiguousarray(x[b][sel, :].T).astype(NPBF)
        m = {"xT": xTc, "wqT": wqT, "wkT": wkT, "wvT": wvT, "woT": woT,
             "bq": bqr, "bk": bkr, "bv": bvr, "bo": bor}
        if causal:
            m["cmb"] = _mask01(p)
        in_maps.append(m)

    global LAST_RESULT
    res = run_bass_kernel_spmd(nc, in_maps, list(range(NC)), trace=TRACE)
    LAST_RESULT = res
    out = np.empty((B, S, D), dtype=np.float32)
    for c in range(NC):
        b, p = divmod(c, 4)
        sel = sel_tokens(p)
        out[b, sel, :] = np.asarray(res.results[c]["outT"], dtype=np.float32).T
    return out



# revision 2
# speedup vs baseline: 1.6474x; 1.6474x over previous
"""Trainium2 Bass kernel for nn_MultiHeadAttention (B=2, S=2048, D=1024, H=16, causal).

Sharding across 8 NeuronCores -- NO on-device collective:
  - Core c owns batch b=c//4 and head-group g=c%4 (4 heads).  Wq/Wk/Wv are
    column-sharded (256 features per core), Wo is row-sharded; each core
    emits a PARTIAL output projection over the full 2048 tokens and the
    host sums the 4 partials per batch at unshard time.  This removes the
    AllGather + barrier that cost ~190us in the token-sharded design.
  - Everything on-chip is bf16 (PSUM accumulation fp32); softmax scale is
    folded into Wk/bk on the host; biases are rank-1 matmuls into PSUM.
  - Pipelined per 512-token chunk tc: project K/V/Q for chunk tc, then
    attention for query chunk tc (keys 0..512*tc+511 already projected),
    then the output projection for those tokens -- DMA-in overlaps the
    first projections and every engine stays fed.
  - Attention per head pair (feature block = 128 partitions): score
    matmuls for the two heads are row-tiled (partitions 0:64 / 64:128)
    into one 2-bank PSUM tile so they run concurrently; one wide exp
    covers both heads.  Diagonal key blocks are width-trimmed to
    512-128*o columns and masked with a single constant [128,2,128]
    staircase (beyond 128 columns the causal mask is all-ones).  ctx
    accumulates in PSUM across key blocks with a 65th stationary V column
    (= softmax denominator); reciprocal is exp(-ln(d)) on the scalar
    engine, replicated to 64 partitions by a rank-1 matmul.
"""
import numpy as np
import ml_dtypes

import concourse.bass as bass
import concourse.bacc as bacc
import concourse.mybir as mybir
import concourse.tile as tile
from concourse.bass_utils import run_bass_kernel_spmd

B, S, D, H, HD = 2, 2048, 1024, 16, 64
NC = 8
P = 128
F32 = mybir.dt.float32
BF = mybir.dt.bfloat16
NPBF = ml_dtypes.bfloat16

TRACE = False        # set True (e.g. from test.py) to capture an NTFF profile
LAST_RESULT = None   # BassKernelResults of the most recent kernel() call

_ACT_PATCHED = False


def _patch_act_tables():
    """Steer Bacc's act-table-load pass to the combined natural_log+exp
    set so a kernel using both Exp and Ln takes ONE table load instead of
    alternating between table sets (~1.3us per switch on scalar)."""
    global _ACT_PATCHED
    if _ACT_PATCHED:
        return
    import concourse.bacc as _bacc
    _orig = _bacc.get_activation_tables

    def _filtered(arch):
        t = _orig(arch)
        fexp = mybir.ActivationFunctionType.Exp
        fln = mybir.ActivationFunctionType.Ln
        out = {}
        for name, fns in t.items():
            if name != "natural_log_exp_and_others" and (
                    fexp in fns or fln in fns):
                fns = fns - {fexp, fln}
            out[name] = fns
        return out

    _bacc.get_activation_tables = _filtered
    _ACT_PATCHED = True


def _emit(causal: bool):
    nc = bacc.Bacc(trn_type="TRN2", num_devices=NC)
    fexp = mybir.ActivationFunctionType.Exp
    fln = mybir.ActivationFunctionType.Ln
    _patch_act_tables()

    xT = nc.dram_tensor("xT", [D, S], BF, kind="ExternalInput")
    wqT = nc.dram_tensor("wqT", [D, 256], BF, kind="ExternalInput")
    wkT = nc.dram_tensor("wkT", [D, 256], BF, kind="ExternalInput")
    wvT = nc.dram_tensor("wvT", [D, 256], BF, kind="ExternalInput")
    woT = nc.dram_tensor("woT", [256, D], BF, kind="ExternalInput")
    bq_d = nc.dram_tensor("bq", [1, 256], BF, kind="ExternalInput")
    bk_d = nc.dram_tensor("bk", [1, 256], BF, kind="ExternalInput")
    bv_d = nc.dram_tensor("bv", [1, 256], BF, kind="ExternalInput")
    if causal:
        cm_d = nc.dram_tensor("cm", [P, 2, P], BF, kind="ExternalInput")
    outT = nc.dram_tensor("outT", [D, S], BF, kind="ExternalOutput")

    with tile.TileContext(nc) as tc, \
         tc.tile_pool(name="const", bufs=1) as const, \
         tc.tile_pool(name="big", bufs=1) as big, \
         tc.tile_pool(name="oio", bufs=3) as oio, \
         tc.tile_pool(name="vio", bufs=2) as vio, \
         tc.tile_pool(name="ex", bufs=4) as ex, \
         tc.tile_pool(name="sm", bufs=2) as sm, \
         tc.tile_pool(name="ps_sc", bufs=2, space="PSUM") as ps_sc, \
         tc.tile_pool(name="ps_ctx", bufs=2, space="PSUM") as ps_ctx, \
         tc.tile_pool(name="ps_w", bufs=2, space="PSUM") as ps_w:

        # ---------- constants / inputs ----------
        ones = const.tile([P, 512], BF)
        nc.gpsimd.memset(ones[:], 1.0)
        bq_sb = const.tile([1, 256], BF)
        nc.sync.dma_start(bq_sb[:], bq_d[:])
        bk_sb = const.tile([1, 256], BF)
        nc.sync.dma_start(bk_sb[:], bk_d[:])
        bv_sb = const.tile([1, 256], BF)
        nc.sync.dma_start(bv_sb[:], bv_d[:])
        if causal:
            cm_sb = const.tile([P, 2, P], BF)
            nc.sync.dma_start(cm_sb[:], cm_d[:])

        wk_sb = big.tile([P, 8, 256], BF)
        wv_sb = big.tile([P, 8, 256], BF)
        wq_sb = big.tile([P, 8, 256], BF)
        wo_sb = big.tile([P, 2, D], BF)
        xt_sb = big.tile([P, 8, S], BF)
        kt_sb = big.tile([P, 2, S], BF)
        qt_sb = big.tile([P, 2, S], BF)
        v_sb = big.tile([P, 16, 4, 65], BF)
        ctx_sb = big.tile([P, 2, S], BF)

        wkr = wkT.rearrange("(o p) f -> p o f", p=P)
        wvr = wvT.rearrange("(o p) f -> p o f", p=P)
        wqr = wqT.rearrange("(o p) f -> p o f", p=P)
        wor = woT.rearrange("(o p) f -> p o f", p=P)
        xr = xT.rearrange("(o p) t -> p o t", p=P)
        outr = outT.rearrange("(o p) t -> p o t", p=P)

        # DMA order: wk first, then x chunk 0 (first K matmuls start early),
        # then wv/wq, remaining x, wo.
        for kt in range(8):
            nc.sync.dma_start(wk_sb[:, kt, :], wkr[:, kt, :])
        for kt in range(8):
            nc.sync.dma_start(xt_sb[:, kt, 0:512], xr[:, kt, 0:512])
        for kt in range(8):
            nc.sync.dma_start(wv_sb[:, kt, :], wvr[:, kt, :])
            nc.sync.dma_start(wq_sb[:, kt, :], wqr[:, kt, :])
        for tcx in range(1, 4):
            for kt in range(8):
                nc.sync.dma_start(xt_sb[:, kt, 512 * tcx:512 * tcx + 512],
                                  xr[:, kt, 512 * tcx:512 * tcx + 512])
        for kt in range(2):
            nc.sync.dma_start(wo_sb[:, kt, :], wor[:, kt, :])

        def proj_chunk(tc_i):
            t0 = 512 * tc_i
            # K^T and Q^T: out[feat, tok], feature block fb == head pair
            for w_sb, b_sb, sink in ((wk_sb, bk_sb, kt_sb),
                                     (wq_sb, bq_sb, qt_sb)):
                for fb in range(2):
                    pt = ps_w.tile([P, 512], F32, tag="psw")
                    for kt in range(8):
                        nc.tensor.matmul(
                            pt[:], w_sb[:, kt, 128 * fb:128 * fb + 128],
                            xt_sb[:, kt, t0:t0 + 512],
                            start=(kt == 0), stop=False)
                    nc.tensor.matmul(
                        pt[:], b_sb[0:1, 128 * fb:128 * fb + 128],
                        ones[0:1, 0:512], start=False, stop=True)
                    nc.vector.tensor_copy(sink[:, fb, t0:t0 + 512], pt[:])
            # V: out[tok, feat] per 128-token block; 65th col = ones
            for tb in range(4):
                jb = 4 * tc_i + tb
                pt = ps_w.tile([P, 512], F32, tag="psw")
                for kt in range(8):
                    nc.tensor.matmul(
                        pt[:, 0:256],
                        xt_sb[:, kt, t0 + 128 * tb:t0 + 128 * tb + 128],
                        wv_sb[:, kt, :], start=(kt == 0), stop=False)
                nc.tensor.matmul(
                    pt[:, 0:256], ones[0:1, 0:P], bv_sb[0:1, :],
                    start=False, stop=True)
                vt = vio.tile([P, 4, 65], BF, tag="vio")
                nc.vector.tensor_copy(
                    vt[:, :, 0:64],
                    pt[:, 0:256].rearrange("p (h d) -> p h d", h=4))
                nc.vector.tensor_copy(vt[:, :, 64:65], ones[:, 0:4, None])
                nc.vector.tensor_copy(v_sb[:, jb, :, :], vt[:])

        def attn_pair(tc_i, pair):
            t0 = 512 * tc_i
            jn = 4 * tc_i + 4 if causal else 16
            ctx0 = ps_ctx.tile([P, 512], F32, tag="ctx")
            ctx1 = ps_ctx.tile([P, 512], F32, tag="ctx")
            for j in range(jn):
                o_ = j - 4 * tc_i if causal else -1
                qo = 0 if o_ < 0 else 128 * o_
                wid = 512 - qo
                sc = ps_sc.tile([P, 1024], F32, tag="sc")
                for hh in range(2):
                    nc.tensor.matmul(
                        sc[:, 512 * hh:512 * hh + wid],
                        kt_sb[64 * hh:64 * hh + 64, pair,
                              128 * j:128 * j + 128],
                        qt_sb[64 * hh:64 * hh + 64, pair,
                              t0 + qo:t0 + qo + wid],
                        start=True, stop=True)
                et = ex.tile([P, 2, 512], BF, tag="exp")
                if wid == 512:
                    nc.scalar.activation(et[:, :, :], sc[:, :], fexp)
                else:
                    nc.scalar.activation(
                        et[:, :, 0:wid],
                        sc[:].rearrange("p (s n) -> p s n", s=2)[:, :, 0:wid],
                        fexp)
                if o_ >= 0:
                    nc.vector.tensor_tensor(
                        et[:, :, 0:P], et[:, :, 0:P], cm_sb[:],
                        mybir.AluOpType.mult)
                for hh, ctx in ((0, ctx0), (1, ctx1)):
                    nc.tensor.matmul(
                        ctx[0:65, qo:qo + wid],
                        v_sb[:, j, 2 * pair + hh, :], et[:, hh, 0:wid],
                        start=(j == 0), stop=(j == jn - 1))
            # normalize: recip = exp(-ln(denominator)); even head lands on
            # partitions 0:64 of ctx_sb, odd head moves to 64:128 via a
            # small SBUF->SBUF DMA (cross-partition).
            lnd0 = sm.tile([1, 512], F32, tag="lnd")
            nc.scalar.activation(lnd0[:], ctx0[64:65, 0:512], fln)
            lnd1 = sm.tile([1, 512], F32, tag="lnd")
            nc.scalar.activation(lnd1[:], ctx1[64:65, 0:512], fln)
            rcp0 = sm.tile([1, 512], BF, tag="rcp")
            nc.scalar.activation(rcp0[:], lnd0[:], fexp, scale=-1.0)
            rcp1 = sm.tile([1, 512], BF, tag="rcp")
            nc.scalar.activation(rcp1[:], lnd1[:], fexp, scale=-1.0)
            rep_ps0 = ps_w.tile([P, 512], F32, tag="psw")
            nc.tensor.matmul(rep_ps0[0:64, :], ones[0:1, 0:64],
                             rcp0[0:1, :], start=True, stop=True)
            rep_ps1 = ps_w.tile([P, 512], F32, tag="psw")
            nc.tensor.matmul(rep_ps1[0:64, :], ones[0:1, 0:64],
                             rcp1[0:1, :], start=True, stop=True)
            rep0 = sm.tile([64, 512], F32, tag="rep")
            nc.vector.tensor_copy(rep0[:], rep_ps0[0:64, :])
            rep1 = sm.tile([64, 512], F32, tag="rep")
            nc.vector.tensor_copy(rep1[:], rep_ps1[0:64, :])
            nc.vector.tensor_tensor(
                ctx_sb[0:64, pair, t0:t0 + 512], ctx0[0:64, :],
                rep0[:], mybir.AluOpType.mult)
            ctmp = sm.tile([64, 512], BF, tag="ctmp")
            nc.vector.tensor_tensor(
                ctmp[:], ctx1[0:64, :], rep1[:], mybir.AluOpType.mult)
            nc.sync.dma_start(ctx_sb[64:128, pair, t0:t0 + 512], ctmp[:])

        def outproj_chunk(tc_i):
            t0 = 512 * tc_i
            for m in range(8):
                pt = ps_w.tile([P, 512], F32, tag="psw")
                for kt in range(2):
                    nc.tensor.matmul(
                        pt[:], wo_sb[:, kt, 128 * m:128 * m + 128],
                        ctx_sb[:, kt, t0:t0 + 512],
                        start=(kt == 0), stop=(kt == 1))
                t = oio.tile([P, 512], BF, tag="oio")
                nc.vector.tensor_copy(t[:], pt[:])
                nc.sync.dma_start(outr[:, m, t0:t0 + 512], t[:])

        if causal:
            for tc_i in range(4):
                proj_chunk(tc_i)
                attn_pair(tc_i, 0)
                attn_pair(tc_i, 1)
                outproj_chunk(tc_i)
        else:
            for tc_i in range(4):
                proj_chunk(tc_i)
            for tc_i in range(4):
                attn_pair(tc_i, 0)
                attn_pair(tc_i, 1)
                outproj_chunk(tc_i)

    nc.compile()
    return nc


_CACHE = {}


def _get_nc(causal: bool):
    if causal not in _CACHE:
        _CACHE[causal] = _emit(causal)
    return _CACHE[causal]


def kernel(**inputs):
    x = np.asarray(inputs["x"], dtype=np.float32)
    Wq = np.asarray(inputs["Wq"], dtype=np.float32)
    bq = np.asarray(inputs["bq"], dtype=np.float32)
    Wk = np.asarray(inputs["Wk"], dtype=np.float32)
    bk = np.asarray(inputs["bk"], dtype=np.float32)
    Wv = np.asarray(inputs["Wv"], dtype=np.float32)
    bv = np.asarray(inputs["bv"], dtype=np.float32)
    Wo = np.asarray(inputs["Wo"], dtype=np.float32)
    bo = np.asarray(inputs["bo"], dtype=np.float32)
    causal = bool(int(np.asarray(inputs["enable_causal"])))

    scale = np.float32(1.0 / np.sqrt(HD))
    xTb = [np.ascontiguousarray(x[b].T).astype(NPBF) for b in range(B)]
    cm = np.ascontiguousarray(np.broadcast_to(
        (np.arange(P)[:, None] <= np.arange(P)[None, :])
        .astype(np.float32)[:, None, :], (P, 2, P))).astype(NPBF)

    nc = _get_nc(causal)
    in_maps = []
    for c in range(NC):
        b, g = divmod(c, 4)
        f0 = 256 * g
        m = {"xT": xTb[b],
             "wqT": np.ascontiguousarray(Wq[f0:f0 + 256, :].T).astype(NPBF),
             "wkT": np.ascontiguousarray(
                 (Wk[f0:f0 + 256, :] * scale).T).astype(NPBF),
             "wvT": np.ascontiguousarray(Wv[f0:f0 + 256, :].T).astype(NPBF),
             "woT": np.ascontiguousarray(Wo[:, f0:f0 + 256].T).astype(NPBF),
             "bq": bq[f0:f0 + 256].reshape(1, 256).astype(NPBF),
             "bk": (bk[f0:f0 + 256] * scale).reshape(1, 256).astype(NPBF),
             "bv": bv[f0:f0 + 256].reshape(1, 256).astype(NPBF)}
        if causal:
            m["cm"] = cm
        in_maps.append(m)

    global LAST_RESULT
    res = run_bass_kernel_spmd(nc, in_maps, list(range(NC)), trace=TRACE)
    LAST_RESULT = res
    out = np.zeros((B, S, D), dtype=np.float32)
    for c in range(NC):
        b = c // 4
        out[b] += np.asarray(res.results[c]["outT"], dtype=np.float32).T
    out += bo[None, None, :]
    return out


# revision 4
# speedup vs baseline: 2.0549x; 1.2474x over previous
"""Trainium2 Bass kernel for nn_MultiHeadAttention (B=2, S=2048, D=1024, H=16, causal).

Sharding across 8 NeuronCores -- NO on-device collective:
  - Core c owns batch b=c//4 and head-group g=c%4 (4 heads).  Wq/Wk/Wv are
    column-sharded (256 features per core), Wo is row-sharded; each core
    emits a PARTIAL output projection over the full 2048 tokens and the
    host sums the 4 partials per batch at unshard time.  This removes the
    AllGather + barrier that cost ~190us in the token-sharded design.
  - Everything on-chip is bf16 (PSUM accumulation fp32); softmax scale is
    folded into Wk/bk on the host.  K/Q biases are per-partition
    tensor_scalar adds fused into the PSUM->SBUF copy; V bias is a rank-1
    matmul into PSUM.
  - Pipelined per 512-token chunk tc: project K/V/Q for chunk tc, run
    attention for query chunk tc (keys 0..512*tc+511), then project chunk
    tc+1 BEFORE the output projection of tc so the softmax-normalize tail
    never stalls the PE.
  - Attention per head pair (feature block = 128 partitions): score
    matmuls for the two heads are row-tiled (partitions 0:64 / 64:128)
    into one 2-bank PSUM tile so they run concurrently; one wide exp
    covers both heads; emission is software-pipelined (scores of block
    j+1 issue before ctx of block j) so the PE never waits on the scalar
    engine's exp.  Diagonal key blocks are width-trimmed to 512-128*o
    columns; the causal mask restricted to the computed window is the
    same [128,2,128] staircase for every block (beyond 128 columns it is
    all-ones).  ctx accumulates in PSUM across key blocks with a 65th
    stationary V column (= softmax denominator); ctx is copied raw to
    SBUF right away to free the PSUM banks for the next pair, and the
    normalize chain (recip = exp(-ln(d)) on scalar, rank-1 replicate,
    DVE mult) runs off the critical path.
"""
import numpy as np
import ml_dtypes

import concourse.bass as bass
import concourse.bacc as bacc
import concourse.mybir as mybir
import concourse.tile as tile
from concourse.bass_utils import run_bass_kernel_spmd

B, S, D, H, HD = 2, 2048, 1024, 16, 64
NC = 8
P = 128
F32 = mybir.dt.float32
BF = mybir.dt.bfloat16
NPBF = ml_dtypes.bfloat16

TRACE = False        # set True (e.g. from test.py) to capture an NTFF profile
LAST_RESULT = None   # BassKernelResults of the most recent kernel() call

_ACT_PATCHED = False


def _patch_act_tables():
    """Steer Bacc's act-table-load pass to the combined natural_log+exp
    set so a kernel using both Exp and Ln takes ONE table load instead of
    alternating between table sets (~1.3us per switch on scalar)."""
    global _ACT_PATCHED
    if _ACT_PATCHED:
        return
    import concourse.bacc as _bacc
    _orig = _bacc.get_activation_tables

    def _filtered(arch):
        t = _orig(arch)
        fexp = mybir.ActivationFunctionType.Exp
        fln = mybir.ActivationFunctionType.Ln
        out = {}
        for name, fns in t.items():
            if name != "natural_log_exp_and_others" and (
                    fexp in fns or fln in fns):
                fns = fns - {fexp, fln}
            out[name] = fns
        return out

    _bacc.get_activation_tables = _filtered
    _ACT_PATCHED = True


def _emit(causal: bool):
    nc = bacc.Bacc(trn_type="TRN2", num_devices=NC)
    fexp = mybir.ActivationFunctionType.Exp
    fln = mybir.ActivationFunctionType.Ln
    _patch_act_tables()

    xT = nc.dram_tensor("xT", [D, S], BF, kind="ExternalInput")
    wqT = nc.dram_tensor("wqT", [D, 256], BF, kind="ExternalInput")
    wkT = nc.dram_tensor("wkT", [D, 256], BF, kind="ExternalInput")
    wvT = nc.dram_tensor("wvT", [D, 256], BF, kind="ExternalInput")
    woT = nc.dram_tensor("woT", [256, D], BF, kind="ExternalInput")
    bqc_d = nc.dram_tensor("bqc", [P, 2], F32, kind="ExternalInput")
    bkc_d = nc.dram_tensor("bkc", [P, 2], F32, kind="ExternalInput")
    bv_d = nc.dram_tensor("bv", [1, 256], BF, kind="ExternalInput")
    if causal:
        cm_d = nc.dram_tensor("cm", [P, 2, P], BF, kind="ExternalInput")
    outT = nc.dram_tensor("outT", [D, S], BF, kind="ExternalOutput")

    with tile.TileContext(nc) as tc, \
         tc.tile_pool(name="const", bufs=1) as const, \
         tc.tile_pool(name="big", bufs=1) as big, \
         tc.tile_pool(name="oio", bufs=3) as oio, \
         tc.tile_pool(name="ex", bufs=4) as ex, \
         tc.tile_pool(name="u", bufs=4) as up, \
         tc.tile_pool(name="sm", bufs=2) as sm, \
         tc.tile_pool(name="ps_sc", bufs=2, space="PSUM") as ps_sc, \
         tc.tile_pool(name="ps_ctx", bufs=2, space="PSUM") as ps_ctx, \
         tc.tile_pool(name="ps_w", bufs=2, space="PSUM") as ps_w:

        # ---------- constants / inputs ----------
        ones = const.tile([1, P], BF)
        nc.gpsimd.memset(ones[:], 1.0)
        bqc_sb = const.tile([P, 2], F32)
        nc.scalar.dma_start(bqc_sb[:], bqc_d[:])
        bkc_sb = const.tile([P, 2], F32)
        nc.scalar.dma_start(bkc_sb[:], bkc_d[:])
        bv_sb = const.tile([1, 256], BF)
        nc.scalar.dma_start(bv_sb[:], bv_d[:])
        if causal:
            cm_sb = const.tile([P, 2, P], BF)
            nc.scalar.dma_start(cm_sb[:], cm_d[:])

        wk_sb = big.tile([P, 8, 256], BF)
        wv_sb = big.tile([P, 8, 256], BF)
        wq_sb = big.tile([P, 8, 256], BF)
        wo_sb = big.tile([P, 2, D], BF)
        xt_sb = big.tile([P, 8, S], BF)
        kt_sb = big.tile([P, 2, S], BF)
        qt_sb = big.tile([P, 2, S], BF)
        v_sb = big.tile([P, 16, 4, 65], BF)
        ctx_sb = big.tile([P, 2, S], BF)
        nc.gpsimd.memset(v_sb[:, :, :, 64:65], 1.0)

        wkr = wkT.rearrange("(o p) f -> p o f", p=P)
        wvr = wvT.rearrange("(o p) f -> p o f", p=P)
        wqr = wqT.rearrange("(o p) f -> p o f", p=P)
        wor = woT.rearrange("(o p) f -> p o f", p=P)
        xr = xT.rearrange("(o p) t -> p o t", p=P)
        outr = outT.rearrange("(o p) t -> p o t", p=P)

        # DMA order on sync queue: wk, x chunk 0 (first K matmuls start
        # early), wv, wq, rest of x, wo.
        nc.sync.dma_start(wk_sb[:], wkr[:])
        for kt in range(8):
            nc.sync.dma_start(xt_sb[:, kt, 0:512], xr[:, kt, 0:512])
        nc.sync.dma_start(wv_sb[:], wvr[:])
        nc.sync.dma_start(wq_sb[:], wqr[:])
        for kt in range(8):
            nc.sync.dma_start(xt_sb[:, kt, 512:2048], xr[:, kt, 512:2048])
        nc.sync.dma_start(wo_sb[:], wor[:])

        def proj_chunk(tc_i):
            t0 = 512 * tc_i
            # K^T and Q^T: out[feat, tok], feature block fb == head pair;
            # bias is a per-partition scalar add fused into the copy.
            for w_sb, b_sb, sink in ((wk_sb, bkc_sb, kt_sb),
                                     (wq_sb, bqc_sb, qt_sb)):
                for fb in range(2):
                    pt = ps_w.tile([P, 512], F32, tag="psw")
                    for kt in range(8):
                        nc.tensor.matmul(
                            pt[:], w_sb[:, kt, 128 * fb:128 * fb + 128],
                            xt_sb[:, kt, t0:t0 + 512],
                            start=(kt == 0), stop=(kt == 7))
                    nc.vector.tensor_scalar_add(
                        sink[:, fb, t0:t0 + 512], pt[:], b_sb[:, fb:fb + 1])
            # V: out[tok, feat] per 128-token block (65th col pre-set to 1)
            for tb in range(4):
                jb = 4 * tc_i + tb
                pt = ps_w.tile([P, 512], F32, tag="psw")
                for kt in range(8):
                    nc.tensor.matmul(
                        pt[:, 0:256],
                        xt_sb[:, kt, t0 + 128 * tb:t0 + 128 * tb + 128],
                        wv_sb[:, kt, :], start=(kt == 0), stop=False)
                nc.tensor.matmul(
                    pt[:, 0:256], ones[0:1, 0:P], bv_sb[0:1, :],
                    start=False, stop=True)
                nc.vector.tensor_copy(
                    v_sb[:, jb, :, 0:64],
                    pt[:, 0:256].rearrange("p (h d) -> p h d", h=4))

        def attn_pair(tc_i, pair):
            """Scores+exp+ctx for one head pair; software-pipelined so the
            ctx matmul of block j issues after the scores of block j+1.
            Returns the raw [65,512] SBUF copies (u0, u1) for deferred
            normalization."""
            t0 = 512 * tc_i
            jn = 4 * tc_i + 4 if causal else 16
            ctx0 = ps_ctx.tile([P, 512], F32, tag="ctx")
            ctx1 = ps_ctx.tile([P, 512], F32, tag="ctx")
            prev = None

            def emit_ctx(pj, pet, pqo, pwid):
                for hh, ctx in ((0, ctx0), (1, ctx1)):
                    nc.tensor.matmul(
                        ctx[0:65, pqo:pqo + pwid],
                        v_sb[:, pj, 2 * pair + hh, :], pet[:, hh, 0:pwid],
                        start=(pj == 0), stop=(pj == jn - 1))

            for j in range(jn):
                o_ = j - 4 * tc_i if causal else -1
                qo = 0 if o_ < 0 else 128 * o_
                wid = 512 - qo
                sc = ps_sc.tile([P, 1024], F32, tag="sc")
                for hh in range(2):
                    nc.tensor.matmul(
                        sc[:, 512 * hh:512 * hh + wid],
                        kt_sb[64 * hh:64 * hh + 64, pair,
                              128 * j:128 * j + 128],
                        qt_sb[64 * hh:64 * hh + 64, pair,
                              t0 + qo:t0 + qo + wid],
                        start=True, stop=True)
                et = ex.tile([P, 2, 512], BF, tag="exp")
                if wid == 512:
                    nc.scalar.activation(et[:, :, :], sc[:, :], fexp)
                else:
                    nc.scalar.activation(
                        et[:, :, 0:wid],
                        sc[:].rearrange("p (s n) -> p s n", s=2)[:, :, 0:wid],
                        fexp)
                if o_ >= 0:
                    nc.vector.tensor_tensor(
                        et[:, :, 0:P], et[:, :, 0:P], cm_sb[:],
                        mybir.AluOpType.mult)
                if prev is not None:
                    emit_ctx(*prev)
                prev = (j, et, qo, wid)
            emit_ctx(*prev)
            u0 = up.tile([65, 512], F32, tag="u")
            nc.vector.tensor_copy(u0[:], ctx0[0:65, :])
            u1 = up.tile([65, 512], F32, tag="u")
            nc.vector.tensor_copy(u1[:], ctx1[0:65, :])
            return u0, u1

        def norm_pair(tc_i, pair, u0, u1):
            """recip = exp(-ln(denominator)); even head lands on partitions
            0:64 of ctx_sb, odd head moves to 64:128 via a small
            SBUF->SBUF DMA (cross-partition)."""
            t0 = 512 * tc_i
            lnd0 = sm.tile([1, 512], F32, tag="lnd")
            nc.scalar.activation(lnd0[:], u0[64:65, 0:512], fln)
            lnd1 = sm.tile([1, 512], F32, tag="lnd")
            nc.scalar.activation(lnd1[:], u1[64:65, 0:512], fln)
            rcp0 = sm.tile([1, 512], BF, tag="rcp")
            nc.scalar.activation(rcp0[:], lnd0[:], fexp, scale=-1.0)
            rcp1 = sm.tile([1, 512], BF, tag="rcp")
            nc.scalar.activation(rcp1[:], lnd1[:], fexp, scale=-1.0)
            rep_ps0 = ps_w.tile([P, 512], F32, tag="psw")
            nc.tensor.matmul(rep_ps0[0:64, :], ones[0:1, 0:64],
                             rcp0[0:1, :], start=True, stop=True)
            rep_ps1 = ps_w.tile([P, 512], F32, tag="psw")
            nc.tensor.matmul(rep_ps1[0:64, :], ones[0:1, 0:64],
                             rcp1[0:1, :], start=True, stop=True)
            rep0 = sm.tile([64, 512], F32, tag="rep")
            nc.vector.tensor_copy(rep0[:], rep_ps0[0:64, :])
            rep1 = sm.tile([64, 512], F32, tag="rep")
            nc.vector.tensor_copy(rep1[:], rep_ps1[0:64, :])
            nc.vector.tensor_tensor(
                ctx_sb[0:64, pair, t0:t0 + 512], u0[0:64, :],
                rep0[:], mybir.AluOpType.mult)
            ctmp = sm.tile([64, 512], BF, tag="ctmp")
            nc.vector.tensor_tensor(
                ctmp[:], u1[0:64, :], rep1[:], mybir.AluOpType.mult)
            nc.sync.dma_start(ctx_sb[64:128, pair, t0:t0 + 512], ctmp[:])

        def outproj_chunk(tc_i):
            t0 = 512 * tc_i
            for m in range(8):
                pt = ps_w.tile([P, 512], F32, tag="psw")
                for kt in range(2):
                    nc.tensor.matmul(
                        pt[:], wo_sb[:, kt, 128 * m:128 * m + 128],
                        ctx_sb[:, kt, t0:t0 + 512],
                        start=(kt == 0), stop=(kt == 1))
                t = oio.tile([P, 512], BF, tag="oio")
                nc.vector.tensor_copy(t[:], pt[:])
                nc.sync.dma_start(outr[:, m, t0:t0 + 512], t[:])

        if causal:
            proj_chunk(0)
            for tc_i in range(4):
                ua = attn_pair(tc_i, 0)
                ub = attn_pair(tc_i, 1)
                norm_pair(tc_i, 0, *ua)
                norm_pair(tc_i, 1, *ub)
                if tc_i < 3:
                    proj_chunk(tc_i + 1)
                outproj_chunk(tc_i)
        else:
            for tc_i in range(4):
                proj_chunk(tc_i)
            for tc_i in range(4):
                ua = attn_pair(tc_i, 0)
                ub = attn_pair(tc_i, 1)
                norm_pair(tc_i, 0, *ua)
                norm_pair(tc_i, 1, *ub)
                outproj_chunk(tc_i)

    nc.compile()
    return nc


_CACHE = {}


def _get_nc(causal: bool):
    if causal not in _CACHE:
        _CACHE[causal] = _emit(causal)
    return _CACHE[causal]


def kernel(**inputs):
    x = np.asarray(inputs["x"], dtype=np.float32)
    Wq = np.asarray(inputs["Wq"], dtype=np.float32)
    bq = np.asarray(inputs["bq"], dtype=np.float32)
    Wk = np.asarray(inputs["Wk"], dtype=np.float32)
    bk = np.asarray(inputs["bk"], dtype=np.float32)
    Wv = np.asarray(inputs["Wv"], dtype=np.float32)
    bv = np.asarray(inputs["bv"], dtype=np.float32)
    Wo = np.asarray(inputs["Wo"], dtype=np.float32)
    bo = np.asarray(inputs["bo"], dtype=np.float32)
    causal = bool(int(np.asarray(inputs["enable_causal"])))

    scale = np.float32(1.0 / np.sqrt(HD))
    xTb = [np.ascontiguousarray(x[b].T).astype(NPBF) for b in range(B)]
    cm = np.ascontiguousarray(np.broadcast_to(
        (np.arange(P)[:, None] <= np.arange(P)[None, :])
        .astype(np.float32)[:, None, :], (P, 2, P))).astype(NPBF)

    nc = _get_nc(causal)
    in_maps = []
    for c in range(NC):
        b, g = divmod(c, 4)
        f0 = 256 * g
        m = {"xT": xTb[b],
             "wqT": np.ascontiguousarray(Wq[f0:f0 + 256, :].T).astype(NPBF),
             "wkT": np.ascontiguousarray(
                 (Wk[f0:f0 + 256, :] * scale).T).astype(NPBF),
             "wvT": np.ascontiguousarray(Wv[f0:f0 + 256, :].T).astype(NPBF),
             "woT": np.ascontiguousarray(Wo[:, f0:f0 + 256].T).astype(NPBF),
             "bqc": np.ascontiguousarray(
                 bq[f0:f0 + 256].reshape(2, P).T).astype(np.float32),
             "bkc": np.ascontiguousarray(
                 (bk[f0:f0 + 256] * scale).reshape(2, P).T).astype(np.float32),
             "bv": bv[f0:f0 + 256].reshape(1, 256).astype(NPBF)}
        if causal:
            m["cm"] = cm
        in_maps.append(m)

    global LAST_RESULT
    res = run_bass_kernel_spmd(nc, in_maps, list(range(NC)), trace=TRACE)
    LAST_RESULT = res
    out = np.zeros((B, S, D), dtype=np.float32)
    for c in range(NC):
        b = c // 4
        out[b] += np.asarray(res.results[c]["outT"], dtype=np.float32).T
    out += bo[None, None, :]
    return out
